# revision 23
# baseline (speedup 1.0000x reference)
"""Per-pixel adaptive 5x5 conv (KPN) for Trainium2, 8-core data parallel.

out[g,h,w] = sum_{i,j} core[g,5i+j,h,w] * frames_pad[g,h+i-2,w+j-2]
with g = flattened (B,N) = 16 image planes; 2 planes per NeuronCore,
fused into one free dim so every elementwise op covers both.

v2 layout (vs v1's parity-copy scheme): each 128-row block stores 516
frame cols (out cols plus the +-2 halo), so ONE frame tile per row
shift i serves all 5 column taps j of its group: the column shift is
folded into the host weight layout (w'[c'] = w[c'-j]) and the PE reads
each product tile at free-dim offset j when accumulating bank b over
cols [b*516+j, b*516+j+512).  Frame DMA drops from 10.6 to 5.3 MB/core.

Engine split (balanced so DVE's 25 muls, ACT's cast chain, and the DMA
queues all land at ~55-60us):
  DVE   - 25 products w_t*f_t (fp16 2x mode, ~2.2us each full tile)
  ACT   - 14 weight dequants (int8 codes -> fp16 copy, 3.6us each)
  DMA   - 11 weight tiles stored as fp16 codes in DRAM and loaded
          directly (no dequant anywhere; costs +0.53 MB of DMA each,
          cheaper than an engine cast while the queues have slack;
          SWDGE casting DMA was measured to cost ~read+write on the
          queues, worse than both)
  PE    - accumulates the 25 product streams into PSUM fp32 via
          matmuls against a stationary (2^-5 * I); 8 banks = [128,512]

Scheduling structure (the DMA queues round-robin ALL in-flight
transfers at packet granularity, so any tile's completion time is
proportional to total bytes in flight — ordering is everything):
  - DMAs are emitted in consumption-deadline order: int8 (ACT-cast)
    tiles ~a group ahead of fp16-direct tiles, since their arrival is
    followed by a serial 3.6us ACT cast before the mul can run.
  - Group 0 runs on quarter tiles ([128,1032], t0/t1 fp16-direct) so
    the first mul starts when ~0.5 MB has landed, not 4 MB.
  - The last group runs in per-image half passes: banks 0-3 get their
    final accumulation a pass early, so their evac (ACT) and the first
    0.5 MB output store overlap the last 5 muls.

Weights are codes of w/2^-5 (int8: clip(round(32 w), -127, 127); fp16
taps keep full precision); the 2^-5 scale is folded into the PE's
stationary identity, so every dequant is a pure cast.

Host layouts:
  fin [5, 128, 4128] fp16: fin[i][p, (img,blk,c)] =
     Fpad[img, blk*128+p+i, c], Fpad = pad(F, rows 2/2, cols 2/2),
     c in [0,516).
  win [25, 128, 4128] int8: win[t][p, (img,blk,c')] =
     clip(round(32*core[img, t, blk*128+p, c'-j]), -127, 127) for
     c'-j in [0,512) else 0, where j = t%5.
  oout [128, 4096] fp16 (host casts to f32).
"""

import os
import sys

import numpy as np

for _p in ("/opt/trn_rl_repo",):
    if _p not in sys.path and os.path.isdir(_p):
        sys.path.insert(0, _p)

K = 5
NCORES = 8
IMGS_PER_CORE = 2
H = W = 512
NBLK = 4          # 128-row blocks per image
C_BLK = 516       # 512 out cols + 4 halo cols (-2..513)
FREE = IMGS_PER_CORE * NBLK * C_BLK   # 4128
HFREE = FREE // 2                     # 2064 = one image
O_FREE = IMGS_PER_CORE * NBLK * W     # 4096
NBANK = 8
BANK = 512
WSCALE = 2.0 ** -5  # int8 weight dequant scale, folded into PE identity

# Dequant engine assignment per tap t = 5*i + j.
DVE_DEQ = frozenset()
FP16_W = (0, 1, 4, 8, 9, 13, 14, 18, 19, 23, 24)  # stored fp16, no dequant
FP16_IDX = {t: n for n, t in enumerate(FP16_W)}
# remaining 15 taps dequant on ACT

_compiled = {}
last_results = None  # BassKernelResults of the most recent run (for test.py)


def _build_nc():
    import concourse.bacc as bacc
    import concourse.mybir as mybir
    from concourse.tile import TileContext

    f16 = mybir.dt.float16
    f32 = mybir.dt.float32
    i8 = mybir.dt.int8

    nc = bacc.Bacc(None, target_bir_lowering=False, debug=False)
    ident = nc.dram_tensor("ident", [128, 128], f16, kind="ExternalInput")
    fin = nc.dram_tensor("fin", [K, 128, FREE], f16, kind="ExternalInput")
    win = nc.dram_tensor("win", [K * K, 128, FREE], i8, kind="ExternalInput")
    win16 = nc.dram_tensor("win16", [len(FP16_W), 128, FREE], f16,
                           kind="ExternalInput")
    oout = nc.dram_tensor("oout", [128, O_FREE], f16, kind="ExternalOutput")

    n_streams = K * K

    with TileContext(nc) as tc:
        with (
            tc.tile_pool(name="ipool", bufs=1) as ipool,
            tc.tile_pool(name="fpool", bufs=3) as fpool,
            tc.tile_pool(name="fhpool", bufs=1) as fhpool,
            tc.tile_pool(name="w8pool", bufs=2) as w8pool,
            tc.tile_pool(name="whpool", bufs=1) as whpool,
            tc.tile_pool(name="wpool", bufs=2) as wpool,
            tc.tile_pool(name="spool", bufs=3) as spool,
            tc.tile_pool(name="shpool", bufs=3) as shpool,
            tc.tile_pool(name="opool", bufs=1) as opool,
            tc.tile_pool(name="ppool", bufs=1, space="PSUM") as ppool,
        ):
            id_t = ipool.tile([128, 128], f16, tag="ident")

            banks = [ppool.tile([128, BANK], f32, tag=f"b{b}",
                                name=f"bank{b}")
                     for b in range(NBANK)]
            osb = opool.tile([128, O_FREE], f16, tag="osb")

            f_tiles = {}
            w8_tiles = {}
            w_tiles = {}
            bank_n = [0] * NBANK

            def pe_acc(tile, j, bank_list, off0):
                # rhs covers out cols of bank b at product offset j
                for lb, b in enumerate(bank_list):
                    s = bank_n[b]
                    bank_n[b] += 1
                    nc.tensor.matmul(
                        out=banks[b][:],
                        lhsT=id_t[:],
                        rhs=tile[:][:, off0 + lb * C_BLK + j:
                                    off0 + lb * C_BLK + j + BANK],
                        start=(s == 0),
                        stop=(s == n_streams - 1),
                    )

            def emit_w(tg, k):
                t = tg * K + k
                if t in FP16_IDX:
                    # fp16 codes straight from DRAM, no dequant step
                    w_t = wpool.tile([128, FREE], f16, tag=f"w{k}",
                                     name=f"wf16_{t}")
                    nc.sync.dma_start(out=w_t[:], in_=win16[FP16_IDX[t]])
                    w_tiles[t] = w_t
                    return
                w8_t = w8pool.tile([128, FREE], i8, tag=f"w8{k}",
                                   name=f"w8_{t}")
                nc.sync.dma_start(out=w8_t[:], in_=win[t])
                w8_tiles[t] = w8_t
                # ACT dequants are emitted here so ACT chases the DMA
                # arrivals a group ahead of the DVE muls; DVE's own
                # dequants are emitted inline in emit_compute.
                if t not in DVE_DEQ:
                    w_t = wpool.tile([128, FREE], f16, tag=f"w{k}",
                                     name=f"wdq{t}")
                    nc.scalar.copy(out=w_t[:], in_=w8_t[:])
                    w_tiles[t] = w_t

            def emit_compute(tg):
                for k in range(K):
                    t = tg * K + k
                    if t in DVE_DEQ:
                        w_t = wpool.tile([128, FREE], f16, tag=f"w{k}",
                                         name=f"wdq{t}")
                        nc.vector.tensor_copy(out=w_t[:],
                                              in_=w8_tiles[t][:])
                        w_tiles[t] = w_t
                    tmp = spool.tile([128, FREE], f16, tag="s")
                    nc.vector.tensor_mul(out=tmp[:], in0=w_tiles[t][:],
                                         in1=f_tiles[tg][:])
                    pe_acc(tmp, k, range(NBANK), 0)

            def emit_compute_last(tg):
                # last group in per-image half passes: banks 0-3 receive
                # their final accumulation one pass early, so their PSUM
                # evac (on ACT, which is idle by then) and the first
                # output store overlap the second half's muls.
                for h in range(2):
                    for k in range(K):
                        t = tg * K + k
                        sl = slice(h * HFREE, (h + 1) * HFREE)
                        tmp = shpool.tile([128, HFREE], f16, tag="sh")
                        nc.vector.tensor_mul(
                            out=tmp[:], in0=w_tiles[t][:][:, sl],
                            in1=f_tiles[tg][:][:, sl])
                        pe_acc(tmp, k, range(4 * h, 4 * h + 4), 0)
                    if h == 0:
                        for b in range(4):
                            nc.scalar.copy(
                                out=osb[:][:, b * BANK:(b + 1) * BANK],
                                in_=banks[b][:])
                        nc.sync.dma_start(out=oout[:, :O_FREE // 2],
                                          in_=osb[:][:, :O_FREE // 2])

            def emit_f(tg):
                f_t = fpool.tile([128, FREE], f16, tag="f",
                                 name=f"fr{tg}")
                nc.sync.dma_start(out=f_t[:], in_=fin[tg])
                f_tiles[tg] = f_t

            # ---- head: group 0 in quarter tiles (2 blocks each) so the
            # first mul starts as soon as ~0.5 MB lands; t0/t1 are
            # fp16-direct so the first muls are DMA-gated only ----
            QT = FREE // 4  # 1032 = 2 blocks
            wq = {}
            fq = []
            for q in range(4):
                fq.append(fhpool.tile([128, QT], f16, tag=f"fq{q}",
                                      name=f"fq{q}"))
            for q in range(4):
                wt = whpool.tile([128, QT], f16, tag=f"wq{q}",
                                 name=f"w0q{q}")
                nc.sync.dma_start(
                    out=wt[:], in_=win16[FP16_IDX[0]][:, q * QT:(q + 1) * QT])
                wq[(0, q)] = wt
                nc.sync.dma_start(
                    out=fq[q][:], in_=fin[0][:, q * QT:(q + 1) * QT])

            # DMAs globally ordered by consumption deadline: ACT-cast
            # (int8) taps get issued ~a group ahead of the fp16-direct
            # taps, since their arrival is followed by a 3.6us serial
            # cast on ACT before the DVE mul can consume them.
            emit_w(0, 2)
            emit_w(0, 3)
            for q in range(4):
                wt = whpool.tile([128, QT], f16, tag=f"wq{q}",
                                 name=f"w1q{q}")
                nc.sync.dma_start(
                    out=wt[:], in_=win16[FP16_IDX[1]][:, q * QT:(q + 1) * QT])
                wq[(1, q)] = wt
            emit_w(1, 0)
            emit_w(1, 1)
            emit_w(0, 4)
            emit_w(1, 2)
            emit_f(1)
            # identity rides the (empty) ACT hwdge ring, keeping its 128
            # tiny descriptors out of the sync ring's ramp window
            nc.scalar.dma_start(out=id_t[:], in_=ident[:])

            # group 0 compute: 20 quarter-muls, no casts anywhere
            for k in range(K):
                for q in range(4):
                    if k in (0, 1):
                        w_ap = wq[(k, q)][:]
                    else:
                        w_ap = w_tiles[k][:][:, q * QT:(q + 1) * QT]
                    tmp = shpool.tile([128, QT], f16, tag="sq")
                    nc.vector.tensor_mul(out=tmp[:], in0=w_ap,
                                         in1=fq[q][:])
                    pe_acc(tmp, k, [2 * q, 2 * q + 1], 0)

            emit_w(2, 0)
            emit_w(1, 3)
            emit_w(2, 1)
            emit_w(1, 4)
            emit_w(2, 2)
            emit_f(2)
            emit_compute(1)
            emit_w(3, 0)
            emit_w(2, 3)
            emit_w(3, 1)
            emit_w(2, 4)
            emit_w(3, 2)
            emit_f(3)
            emit_compute(2)
            emit_w(4, 0)
            emit_w(3, 3)
            emit_w(4, 1)
            emit_w(3, 4)
            emit_w(4, 2)
            emit_f(4)
            emit_compute(3)
            emit_w(4, 3)
            emit_w(4, 4)
            emit_compute_last(K - 1)

            # ---- tail: banks 4-7 PSUM -> SBUF fp16 split across scalar
            # and vector engines, then the remaining output in two
            # quarter chunks so the first can stream while the last
            # evacs run ----
            for b in range(4, NBANK):
                dst = osb[:][:, b * BANK:(b + 1) * BANK]
                if b % 2 == 0:
                    nc.scalar.copy(out=dst, in_=banks[b][:])
                else:
                    nc.vector.tensor_copy(out=dst, in_=banks[b][:])
                if b == 5:
                    nc.sync.dma_start(
                        out=oout[:, 4 * BANK:6 * BANK],
                        in_=osb[:][:, 4 * BANK:6 * BANK])
            nc.sync.dma_start(out=oout[:, 6 * BANK:],
                              in_=osb[:][:, 6 * BANK:])
    nc.finalize()
    return nc


def _host_prep(frames, core):
    """Build per-core in_maps. frames [4,4,1,512,512] f32, core [4,4,25,1,512,512]."""
    G = NCORES * IMGS_PER_CORE  # 16
    F = np.ascontiguousarray(frames.reshape(G, H, W))
    Wc = core.reshape(G, K * K, H, W)

    # frames: pad rows 2/2, cols 2/2 -> [G, 516, 516]
    Fp = np.pad(F, ((0, 0), (2, 2), (2, 2))).astype(np.float16)
    # A[g, i, blk, p, c] = Fp[g, blk*128+p+i, c]
    A = np.empty((G, K, NBLK, 128, C_BLK), np.float16)
    for i in range(K):
        A[:, i] = Fp[:, i:i + H, :].reshape(G, NBLK, 128, C_BLK)
    # fin[core][i, p, (img, blk, c)]
    fprep = np.ascontiguousarray(
        A.reshape(NCORES, IMGS_PER_CORE, K, NBLK, 128, C_BLK)
        .transpose(0, 2, 4, 1, 3, 5))

    # weights: codes of w/2^-5, column-shifted by j so products line up
    # with an aligned frame read; PE reads the product at offset j.
    # int8-rounded codes for the engine-cast taps, full fp16 codes for
    # the direct-load taps.
    w8 = np.clip(np.round(Wc * (1.0 / WSCALE)), -127, 127).astype(np.int8)
    Ws = np.zeros((G, K * K, H, C_BLK), np.int8)
    for j in range(K):
        Ws[:, j::K, :, j:j + W] = w8[:, j::K]
    wprep = np.ascontiguousarray(
        Ws.reshape(NCORES, IMGS_PER_CORE, K * K, NBLK, 128, C_BLK)
        .transpose(0, 2, 4, 1, 3, 5))

    tf = list(FP16_W)
    Wf = np.zeros((G, len(tf), H, C_BLK), np.float16)
    for n, t in enumerate(tf):
        j = t % K
        Wf[:, n, :, j:j + W] = (Wc[:, t] * (1.0 / WSCALE)).astype(np.float16)
    wfprep = np.ascontiguousarray(
        Wf.reshape(NCORES, IMGS_PER_CORE, len(tf), NBLK, 128, C_BLK)
        .transpose(0, 2, 4, 1, 3, 5))

    ident = (np.eye(128) * WSCALE).astype(np.float16)
    in_maps = []
    for c in range(NCORES):
        in_maps.append({
            "ident": ident,
            "fin": fprep[c].reshape(K, 128, FREE),
            "win": wprep[c].reshape(K * K, 128, FREE),
            "win16": wfprep[c].reshape(len(tf), 128, FREE),
        })
    return in_maps


def kernel(frames, core, bias):
    global last_results
    from concourse.bass_utils import run_bass_kernel_spmd

    frames = np.asarray(frames, dtype=np.float32)
    core = np.asarray(core, dtype=np.float32)

    if "nc" not in _compiled:
        _compiled["nc"] = _build_nc()
    nc = _compiled["nc"]

    in_maps = _host_prep(frames, core)
    trace = os.environ.get("KC_TRACE") == "1"
    tmpdir = os.environ.get("KC_TRACE_DIR") or None
    if tmpdir:
        os.makedirs(tmpdir, exist_ok=True)
    res = run_bass_kernel_spmd(nc, in_maps, list(range(NCORES)), trace=trace,
                               tmpdir=tmpdir)
    last_results = res

    G = NCORES * IMGS_PER_CORE
    out = np.empty((G, H, W), np.float32)
    for c in range(NCORES):
        o = res.results[c]["oout"]  # [128, 4096] f16
        ov = o.reshape(128, IMGS_PER_CORE, NBLK, W).astype(np.float32)
        for img in range(IMGS_PER_CORE):
            out[c * IMGS_PER_CORE + img] = (
                ov[:, img].transpose(1, 0, 2).reshape(H, W))
    return out.reshape(4, 4, H, W)


# revision 24
# speedup vs baseline: 1.1340x; 1.1340x over previous
"""Per-pixel adaptive 5x5 conv (KPN) for Trainium2, 8-core data parallel.

out[g,h,w] = sum_{i,j} core[g,5i+j,h,w] * frames_pad[g,h+i-2,w+j-2]
with g = flattened (B,N) = 16 image planes; 2 planes per NeuronCore.

Layout: PARTITION = 64x64 IMAGE PATCH (8x8 grid x 2 images = 128
partitions).  Each partition's free dim holds its patch row-major, so
BOTH tap shifts (i,j) become free-dim offsets into a single halo'd
frame tile fpd[par][p, 68*68]: value (R,C) = f[pr*64+R-2, pc*64+C-2+par].
Two column-parity copies (par=0 for even j, par=1 for odd j) keep every
tap's in1 access 4-byte aligned for DVE fp16 2x mode.  Frames cost
2*1.18 MB of DMA total (the v1 row-block layout needed 5 row-shifted
copies = 10.6 MB; v2 single-parity still needed 5.3 MB).  Weights are
compact [128, 64*64] patch-major, no padding.

Engine split (balanced so DVE's 25 muls, ACT's cast chain, and the DMA
queues all land at ~55-60us):
  DVE   - 25 products w_t * f_shifted (fp16 2x, ~2.2us full tile)
  ACT   - 14 weight dequants (int8 codes -> fp16 copy, 3.6us each)
  DMA   - 11 weight tiles stored as fp16 codes in DRAM, loaded directly
          (no dequant anywhere; +0.5 MB DMA each is cheaper than an
          engine cast while the queues have slack; SWDGE casting DMA
          costs ~read+write on the queues, worse than both)
  PE    - accumulates the 25 product streams into PSUM fp32 via
          matmuls against a stationary (2^-5 * I); 8 banks = [128,512]
          (bank b = patch rows 8b..8b+8)

Scheduling (DMA queues round-robin ALL in-flight transfers at packet
granularity, so any tile's completion time scales with total bytes in
flight — ordering is everything):
  - weight DMAs in consumption-deadline order: int8 (ACT-cast) tiles
    ~a group ahead of fp16-direct tiles (their arrival is followed by
    a serial 3.6us ACT cast before the mul can run)
  - frame patches arrive as 4 row-chunk DMAs into one tile per parity,
    and group 0 runs as [128,16x64] quarter muls chasing the chunks
  - last group runs in per-image-half passes: banks 0-3 get their
    final accumulation a pass early, so their evac (on the
    then-idle ACT) and the first 0.5 MB output store overlap the
    last five muls

Weights are codes of w/2^-5 (int8: clip(round(32w), -127, 127); fp16
taps keep full precision); the 2^-5 scale is folded into the PE's
stationary identity, so every dequant is a pure cast.

Host layouts (per core, p = img*64 + pr*8 + pc):
  fpd [2, 128, 4624] fp16: fpd[par][p, R*68+C] =
      Fpad[img, pr*64+R, pc*64+C+par], Fpad = pad(F, rows 2/2, cols 2/3)
  win [25, 128, 4096] int8, win16 [11, 128, 4096] fp16:
      [t][p, Rp*64+Cp] = codes of core[img, t, pr*64+Rp, pc*64+Cp]
  oout [128, 4096] fp16 (host casts to f32 and reassembles patches).
"""

import os
import sys

import numpy as np

for _p in ("/opt/trn_rl_repo",):
    if _p not in sys.path and os.path.isdir(_p):
        sys.path.insert(0, _p)

K = 5
NCORES = 8
IMGS_PER_CORE = 2
H = W = 512
PB = 64           # patch edge
PG = 8            # patch grid (8x8 per image)
PH = PB + 4       # halo'd patch edge = 68
PATCH = PH * PH   # 4624 elems per partition per parity
FREE = PB * PB    # 4096 product/weight elems per partition
O_FREE = FREE
NBANK = 8
BANK = 512        # = 8 patch rows
WSCALE = 2.0 ** -5  # weight dequant scale, folded into PE identity

# Dequant engine assignment per tap t = 5*i + j.
FP16_W = (0, 1, 4, 8, 9, 13, 14, 18, 19, 23, 24)  # stored fp16, no dequant
FP16_IDX = {t: n for n, t in enumerate(FP16_W)}
# remaining 14 taps dequant on ACT

# fpd row-chunk splits (halo rows): mul quarter q (out rows 16q..16q+16)
# needs patch rows [16q, 16q+20) at i<=4 => cumulative chunks cover it
FP_SPLITS = (0, 20, 36, 52, 68)

_compiled = {}
last_results = None  # BassKernelResults of the most recent run (for test.py)


def _build_nc():
    import concourse.bacc as bacc
    import concourse.mybir as mybir
    from concourse.tile import TileContext

    f16 = mybir.dt.float16
    f32 = mybir.dt.float32
    i8 = mybir.dt.int8

    nc = bacc.Bacc(None, target_bir_lowering=False, debug=False)
    ident = nc.dram_tensor("ident", [128, 128], f16, kind="ExternalInput")
    fpd = nc.dram_tensor("fpd", [2, 128, PATCH], f16, kind="ExternalInput")
    win = nc.dram_tensor("win", [K * K, 128, FREE], i8, kind="ExternalInput")
    win16 = nc.dram_tensor("win16", [len(FP16_W), 128, FREE], f16,
                           kind="ExternalInput")
    oout = nc.dram_tensor("oout", [128, O_FREE], f16, kind="ExternalOutput")

    n_streams = K * K
    QT = FREE // 4   # 1024 = quarter product (16 out rows)
    HF = FREE // 2   # 2048 = half product (32 out rows)

    with TileContext(nc) as tc:
        with (
            tc.tile_pool(name="ipool", bufs=1) as ipool,
            tc.tile_pool(name="fppool", bufs=1) as fppool,
            tc.tile_pool(name="w8pool", bufs=2) as w8pool,
            tc.tile_pool(name="whpool", bufs=1) as whpool,
            tc.tile_pool(name="wpool", bufs=2) as wpool,
            tc.tile_pool(name="spool", bufs=3) as spool,
            tc.tile_pool(name="shpool", bufs=3) as shpool,
            tc.tile_pool(name="opool", bufs=1) as opool,
            tc.tile_pool(name="ppool", bufs=1, space="PSUM") as ppool,
        ):
            id_t = ipool.tile([128, 128], f16, tag="ident")

            banks = [ppool.tile([128, BANK], f32, tag=f"b{b}",
                                name=f"bank{b}")
                     for b in range(NBANK)]
            osb = opool.tile([128, O_FREE], f16, tag="osb")

            fp_t = [fppool.tile([128, PATCH], f16, tag=f"fp{par}",
                                name=f"fp{par}")
                    for par in range(2)]

            w8_tiles = {}
            w_tiles = {}
            bank_n = [0] * NBANK

            def fview(t, r0, nr):
                # frame operand for tap t over out rows [r0, r0+nr):
                # parity copy j%2, rows i+r0.., cols j (even) / j-1 (odd)
                i, j = divmod(t, K)
                par = j & 1
                c0 = j - par
                v = fp_t[par][:].rearrange("p (r c) -> p r c", r=PH, c=PH)
                return v[:, i + r0:i + r0 + nr, c0:c0 + PB]

            def pe_acc(tile, bank_list):
                for lb, b in enumerate(bank_list):
                    s = bank_n[b]
                    bank_n[b] += 1
                    nc.tensor.matmul(
                        out=banks[b][:],
                        lhsT=id_t[:],
                        rhs=tile[:][:, lb * BANK:(lb + 1) * BANK],
                        start=(s == 0),
                        stop=(s == n_streams - 1),
                    )

            def emit_w(tg, k):
                t = tg * K + k
                if t in FP16_IDX:
                    # fp16 codes straight from DRAM, no dequant step
                    w_t = wpool.tile([128, FREE], f16, tag=f"w{k}",
                                     name=f"wf16_{t}")
                    nc.sync.dma_start(out=w_t[:], in_=win16[FP16_IDX[t]])
                    w_tiles[t] = w_t
                    return
                w8_t = w8pool.tile([128, FREE], i8, tag=f"w8{k}",
                                   name=f"w8_{t}")
                nc.sync.dma_start(out=w8_t[:], in_=win[t])
                w8_tiles[t] = w8_t
                # ACT dequant emitted with the DMA so ACT chases arrivals
                w_t = wpool.tile([128, FREE], f16, tag=f"w{k}",
                                 name=f"wdq{t}")
                nc.scalar.copy(out=w_t[:], in_=w8_t[:])
                w_tiles[t] = w_t

            def emit_compute(tg):
                for k in range(K):
                    t = tg * K + k
                    tmp = spool.tile([128, FREE], f16, tag="s")
                    tv = tmp[:].rearrange("p (r c) -> p r c", r=PB, c=PB)
                    wv = w_tiles[t][:].rearrange("p (r c) -> p r c",
                                                 r=PB, c=PB)
                    nc.vector.tensor_mul(out=tv, in0=wv,
                                         in1=fview(t, 0, PB))
                    pe_acc(tmp, range(NBANK))

            def emit_compute_last(tg):
                # per-image-half passes: banks 0-3 finish a pass early;
                # their evac (ACT only -- DVE is still multiplying) and
                # the first output chunk overlap the second pass.
                for h in range(2):
                    for k in range(K):
                        t = tg * K + k
                        tmp = shpool.tile([128, HF], f16, tag="sh")
                        tv = tmp[:].rearrange("p (r c) -> p r c",
                                              r=PB // 2, c=PB)
                        wv = w_tiles[t][:].rearrange(
                            "p (r c) -> p r c", r=PB, c=PB)[
                            :, h * (PB // 2):(h + 1) * (PB // 2), :]
                        nc.vector.tensor_mul(
                            out=tv, in0=wv,
                            in1=fview(t, h * (PB // 2), PB // 2))
                        pe_acc(tmp, range(4 * h, 4 * h + 4))
                    if h == 0:
                        for b in range(4):
                            nc.scalar.copy(
                                out=osb[:][:, b * BANK:(b + 1) * BANK],
                                in_=banks[b][:])
                        nc.sync.dma_start(out=oout[:, :O_FREE // 2],
                                          in_=osb[:][:, :O_FREE // 2])

            def fp_chunk(par, s):
                a, b = FP_SPLITS[s] * PH, FP_SPLITS[s + 1] * PH
                nc.sync.dma_start(out=fp_t[par][:][:, a:b],
                                  in_=fpd[par][:, a:b])

            def wq_dma(t, q):
                wt = whpool.tile([128, QT], f16, tag=f"wq{q}",
                                 name=f"w{t}q{q}")
                nc.sync.dma_start(
                    out=wt[:], in_=win16[FP16_IDX[t]][:, q * QT:(q + 1) * QT])
                return wt

            # ---- head: frame chunks + t0/t1 quarters interleaved so the
            # first quarter-mul starts when ~0.6 MB has landed ----
            wq = {}
            fp_chunk(0, 0)
            wq[(0, 0)] = wq_dma(0, 0)
            fp_chunk(0, 1)
            wq[(0, 1)] = wq_dma(0, 1)
            emit_w(0, 2)
            fp_chunk(0, 2)
            wq[(0, 2)] = wq_dma(0, 2)
            fp_chunk(0, 3)
            wq[(0, 3)] = wq_dma(0, 3)
            emit_w(0, 3)
            fp_chunk(1, 0)
            fp_chunk(1, 1)
            wq[(1, 0)] = wq_dma(1, 0)
            wq[(1, 1)] = wq_dma(1, 1)
            fp_chunk(1, 2)
            fp_chunk(1, 3)
            wq[(1, 2)] = wq_dma(1, 2)
            wq[(1, 3)] = wq_dma(1, 3)
            emit_w(1, 0)
            emit_w(1, 1)
            emit_w(0, 4)
            emit_w(1, 2)
            # identity rides the (empty) ACT hwdge ring, keeping its 128
            # tiny descriptors out of the sync ring's ramp window
            nc.scalar.dma_start(out=id_t[:], in_=ident[:])

            # group 0 compute: 20 quarter-muls chasing the chunk DMAs
            for k in range(K):
                for q in range(4):
                    t = k
                    if k in (0, 1):
                        wv = wq[(k, q)][:].rearrange(
                            "p (r c) -> p r c", r=PB // 4, c=PB)
                    else:
                        wv = w_tiles[k][:].rearrange(
                            "p (r c) -> p r c", r=PB, c=PB)[
                            :, q * (PB // 4):(q + 1) * (PB // 4), :]
                    tmp = shpool.tile([128, QT], f16, tag="sq")
                    tv = tmp[:].rearrange("p (r c) -> p r c",
                                          r=PB // 4, c=PB)
                    nc.vector.tensor_mul(
                        out=tv, in0=wv,
                        in1=fview(t, q * (PB // 4), PB // 4))
                    pe_acc(tmp, [2 * q, 2 * q + 1])

            # steady groups; weight DMAs in deadline order
            emit_w(2, 0)
            emit_w(1, 3)
            emit_w(2, 1)
            emit_w(1, 4)
            emit_w(2, 2)
            emit_compute(1)
            emit_w(3, 0)
            emit_w(2, 3)
            emit_w(3, 1)
            emit_w(2, 4)
            emit_w(3, 2)
            emit_compute(2)
            emit_w(4, 0)
            emit_w(3, 3)
            emit_w(4, 1)
            emit_w(3, 4)
            emit_w(4, 2)
            emit_compute(3)
            emit_w(4, 3)
            emit_w(4, 4)
            emit_compute_last(K - 1)

            # ---- tail: banks 4-7 evac split scalar/vector, output in
            # two quarter chunks ----
            for b in range(4, NBANK):
                dst = osb[:][:, b * BANK:(b + 1) * BANK]
                if b % 2 == 0:
                    nc.scalar.copy(out=dst, in_=banks[b][:])
                else:
                    nc.vector.tensor_copy(out=dst, in_=banks[b][:])
                if b == 5:
                    nc.sync.dma_start(
                        out=oout[:, 4 * BANK:6 * BANK],
                        in_=osb[:][:, 4 * BANK:6 * BANK])
            nc.sync.dma_start(out=oout[:, 6 * BANK:],
                              in_=osb[:][:, 6 * BANK:])
    nc.finalize()
    return nc


def _host_prep(frames, core):
    """Build per-core in_maps. frames [4,4,1,512,512] f32, core [4,4,25,1,512,512]."""
    G = NCORES * IMGS_PER_CORE  # 16
    F = np.ascontiguousarray(frames.reshape(G, H, W))
    Wc = core.reshape(G, K * K, H, W)

    # frame patches, 2 column-parity copies with 2-pixel halo
    Fp = np.pad(F, ((0, 0), (2, 2), (2, 3))).astype(np.float16)
    fpd = np.empty((G, 2, PG * PG, PH, PH), np.float16)
    for par in range(2):
        for pr in range(PG):
            for pc in range(PG):
                fpd[:, par, pr * PG + pc] = Fp[
                    :, pr * PB:pr * PB + PH,
                    pc * PB + par:pc * PB + par + PH]
    fprep = np.ascontiguousarray(
        fpd.reshape(NCORES, IMGS_PER_CORE, 2, PG * PG, PATCH)
        .transpose(0, 2, 1, 3, 4))  # [core, par, img, patch, PATCH]

    # weights, patch-major codes of w/2^-5
    def patchify(a):  # [G, T, 512, 512] -> [core, T, 128, 4096]
        T = a.shape[1]
        b = (a.reshape(G, T, PG, PB, PG, PB)
             .transpose(0, 1, 2, 4, 3, 5)
             .reshape(NCORES, IMGS_PER_CORE, T, PG * PG, FREE)
             .transpose(0, 2, 1, 3, 4))
        return np.ascontiguousarray(b.reshape(NCORES, T, 128, FREE))

    w8 = np.clip(np.round(Wc * (1.0 / WSCALE)), -127, 127).astype(np.int8)
    wprep = patchify(w8)
    tf = list(FP16_W)
    wfprep = patchify((Wc[:, tf] * (1.0 / WSCALE)).astype(np.float16))

    ident = (np.eye(128) * WSCALE).astype(np.float16)
    in_maps = []
    for c in range(NCORES):
        in_maps.append({
            "ident": ident,
            "fpd": fprep[c].reshape(2, 128, PATCH),
            "win": wprep[c],
            "win16": wfprep[c],
        })
    return in_maps


def kernel(frames, core, bias):
    global last_results
    from concourse.bass_utils import run_bass_kernel_spmd

    frames = np.asarray(frames, dtype=np.float32)
    core = np.asarray(core, dtype=np.float32)

    if "nc" not in _compiled:
        _compiled["nc"] = _build_nc()
    nc = _compiled["nc"]

    in_maps = _host_prep(frames, core)
    trace = os.environ.get("KC_TRACE") == "1"
    tmpdir = os.environ.get("KC_TRACE_DIR") or None
    if tmpdir:
        os.makedirs(tmpdir, exist_ok=True)
    res = run_bass_kernel_spmd(nc, in_maps, list(range(NCORES)), trace=trace,
                               tmpdir=tmpdir)
    last_results = res

    G = NCORES * IMGS_PER_CORE
    out = np.empty((G, H, W), np.float32)
    for c in range(NCORES):
        o = res.results[c]["oout"]  # [128, 4096] f16
        ov = (o.reshape(IMGS_PER_CORE, PG, PG, PB, PB).astype(np.float32)
              .transpose(0, 1, 3, 2, 4).reshape(IMGS_PER_CORE, H, W))
        for img in range(IMGS_PER_CORE):
            out[c * IMGS_PER_CORE + img] = ov[img]
    return out.reshape(4, 4, H, W)


# revision 27
# speedup vs baseline: 1.1698x; 1.0316x over previous
"""Per-pixel adaptive 5x5 conv (KPN) for Trainium2, 8-core data parallel.

out[g,h,w] = sum_{i,j} core[g,5i+j,h,w] * frames_pad[g,h+i-2,w+j-2]
with g = flattened (B,N) = 16 image planes; 2 planes per NeuronCore.

Layout: PARTITION = 64x64 IMAGE PATCH (8x8 grid x 2 images = 128
partitions).  Each partition's free dim holds its patch row-major, so
BOTH tap shifts (i,j) become free-dim offsets into a single halo'd
frame tile fpd[par][p, 68*68]: value (R,C) = f[pr*64+R-2, pc*64+C-2+par].
Two column-parity copies (par=0 for even j, par=1 for odd j) keep every
tap's in1 access 4-byte aligned for DVE fp16 2x mode.  Frames cost
2*1.18 MB of DMA total (the v1 row-block layout needed 5 row-shifted
copies = 10.6 MB; v2 single-parity still needed 5.3 MB).  Weights are
compact [128, 64*64] patch-major, no padding.

Engine split (balanced so DVE's 25 muls, ACT's cast chain, and the DMA
queues all land at ~55-60us):
  DVE   - 25 products w_t * f_shifted (fp16 2x, ~2.2us full tile)
  ACT   - 14 weight dequants (int8 codes -> fp16 copy, 3.6us each)
  DMA   - 11 weight tiles stored as fp16 codes in DRAM, loaded directly
          (no dequant anywhere; +0.5 MB DMA each is cheaper than an
          engine cast while the queues have slack; SWDGE casting DMA
          costs ~read+write on the queues, worse than both)
  PE    - accumulates the 25 product streams into PSUM fp32 via
          matmuls against a stationary (2^-5 * I); 8 banks = [128,512]
          (bank b = patch rows 8b..8b+8)

Scheduling (DMA queues round-robin ALL in-flight transfers at packet
granularity, so any tile's completion time scales with total bytes in
flight — ordering is everything):
  - weight DMAs in consumption-deadline order: int8 (ACT-cast) tiles
    ~a group ahead of fp16-direct tiles (their arrival is followed by
    a serial 3.6us ACT cast before the mul can run)
  - frame patches arrive as 4 row-chunk DMAs into one tile per parity,
    and group 0 runs as [128,16x64] quarter muls chasing the chunks
  - last group runs in per-image-half passes: banks 0-3 get their
    final accumulation a pass early, so their evac (on the
    then-idle ACT) and the first 0.5 MB output store overlap the
    last five muls

Weights are codes of w/2^-5 (int8: clip(round(32w), -127, 127); fp16
taps keep full precision); the 2^-5 scale is folded into the PE's
stationary identity, so every dequant is a pure cast.

Host layouts (per core, p = img*64 + pr*8 + pc):
  fpd [2, 128, 4624] fp16: fpd[par][p, R*68+C] =
      Fpad[img, pr*64+R, pc*64+C+par], Fpad = pad(F, rows 2/2, cols 2/3)
  win [25, 128, 4096] int8, win16 [11, 128, 4096] fp16:
      [t][p, Rp*64+Cp] = codes of core[img, t, pr*64+Rp, pc*64+Cp]
  oout [128, 4096] fp16 (host casts to f32 and reassembles patches).
"""

import os
import sys

import numpy as np

for _p in ("/opt/trn_rl_repo",):
    if _p not in sys.path and os.path.isdir(_p):
        sys.path.insert(0, _p)

K = 5
NCORES = 8
IMGS_PER_CORE = 2
H = W = 512
PB = 64           # patch edge
PG = 8            # patch grid (8x8 per image)
PH = PB + 4       # halo'd patch edge = 68
PATCH = PH * PH   # 4624 elems per partition per parity
FREE = PB * PB    # 4096 product/weight elems per partition
O_FREE = FREE
NBANK = 8
BANK = 512        # = 8 patch rows
WSCALE = 2.0 ** -5  # weight dequant scale, folded into PE identity

# Dequant engine assignment per tap t = 5*i + j.  t0/t1 are int8
# quarter tiles cast by DVE in the ramp window (DVE idles on arrivals
# there anyway, and int8 halves their share of the head DMA flood).
FP16_W = (4, 8, 9, 13, 14, 18, 19, 22, 23, 24)  # stored fp16, no dequant
FP16_IDX = {t: n for n, t in enumerate(FP16_W)}
# remaining 13 taps dequant on ACT

# fpd row-chunk splits (halo rows): mul quarter q (out rows 16q..16q+16)
# needs patch rows [16q, 16q+20) at i<=4 => cumulative chunks cover it
FP_SPLITS = (0, 20, 36, 52, 68)

_compiled = {}
last_results = None  # BassKernelResults of the most recent run (for test.py)


def _build_nc():
    import concourse.bacc as bacc
    import concourse.mybir as mybir
    from concourse.tile import TileContext

    f16 = mybir.dt.float16
    f32 = mybir.dt.float32
    i8 = mybir.dt.int8

    nc = bacc.Bacc(None, target_bir_lowering=False, debug=False)
    ident = nc.dram_tensor("ident", [128, 128], f16, kind="ExternalInput")
    fpd = nc.dram_tensor("fpd", [2, 128, PATCH], f16, kind="ExternalInput")
    win = nc.dram_tensor("win", [K * K, 128, FREE], i8, kind="ExternalInput")
    win16 = nc.dram_tensor("win16", [len(FP16_W), 128, FREE], f16,
                           kind="ExternalInput")
    oout = nc.dram_tensor("oout", [128, O_FREE], f16, kind="ExternalOutput")

    n_streams = K * K
    QT = FREE // 4   # 1024 = quarter product (16 out rows)
    HF = FREE // 2   # 2048 = half product (32 out rows)

    with TileContext(nc) as tc:
        with (
            tc.tile_pool(name="ipool", bufs=1) as ipool,
            tc.tile_pool(name="fppool", bufs=1) as fppool,
            tc.tile_pool(name="w8pool", bufs=2) as w8pool,
            tc.tile_pool(name="whpool", bufs=1) as whpool,
            tc.tile_pool(name="wpool", bufs=2) as wpool,
            tc.tile_pool(name="spool", bufs=3) as spool,
            tc.tile_pool(name="shpool", bufs=3) as shpool,
            tc.tile_pool(name="opool", bufs=1) as opool,
            tc.tile_pool(name="ppool", bufs=1, space="PSUM") as ppool,
        ):
            id_t = ipool.tile([128, 128], f16, tag="ident")

            banks = [ppool.tile([128, BANK], f32, tag=f"b{b}",
                                name=f"bank{b}")
                     for b in range(NBANK)]
            osb = opool.tile([128, O_FREE], f16, tag="osb")

            fp_t = [fppool.tile([128, PATCH], f16, tag=f"fp{par}",
                                name=f"fp{par}")
                    for par in range(2)]

            w8_tiles = {}
            w_tiles = {}
            bank_n = [0] * NBANK

            def fview(t, r0, nr):
                # frame operand for tap t over out rows [r0, r0+nr):
                # parity copy j%2, rows i+r0.., cols j (even) / j-1 (odd)
                i, j = divmod(t, K)
                par = j & 1
                c0 = j - par
                v = fp_t[par][:].rearrange("p (r c) -> p r c", r=PH, c=PH)
                return v[:, i + r0:i + r0 + nr, c0:c0 + PB]

            def pe_acc(tile, bank_list):
                for lb, b in enumerate(bank_list):
                    s = bank_n[b]
                    bank_n[b] += 1
                    nc.tensor.matmul(
                        out=banks[b][:],
                        lhsT=id_t[:],
                        rhs=tile[:][:, lb * BANK:(lb + 1) * BANK],
                        start=(s == 0),
                        stop=(s == n_streams - 1),
                    )

            def emit_w(tg, k):
                t = tg * K + k
                if t in FP16_IDX:
                    # fp16 codes straight from DRAM, no dequant step
                    w_t = wpool.tile([128, FREE], f16, tag=f"w{k}",
                                     name=f"wf16_{t}")
                    nc.sync.dma_start(out=w_t[:], in_=win16[FP16_IDX[t]])
                    w_tiles[t] = w_t
                    return
                w8_t = w8pool.tile([128, FREE], i8, tag=f"w8{k}",
                                   name=f"w8_{t}")
                nc.sync.dma_start(out=w8_t[:], in_=win[t])
                w8_tiles[t] = w8_t
                # ACT dequant emitted with the DMA so ACT chases arrivals
                w_t = wpool.tile([128, FREE], f16, tag=f"w{k}",
                                 name=f"wdq{t}")
                nc.scalar.copy(out=w_t[:], in_=w8_t[:])
                w_tiles[t] = w_t

            def emit_compute(tg):
                for k in range(K):
                    t = tg * K + k
                    tmp = spool.tile([128, FREE], f16, tag="s")
                    tv = tmp[:].rearrange("p (r c) -> p r c", r=PB, c=PB)
                    wv = w_tiles[t][:].rearrange("p (r c) -> p r c",
                                                 r=PB, c=PB)
                    nc.vector.tensor_mul(out=tv, in0=wv,
                                         in1=fview(t, 0, PB))
                    pe_acc(tmp, range(NBANK))

            def emit_compute_last(tg):
                # per-image-half passes: banks 0-3 finish a pass early;
                # their evac (ACT only -- DVE is still multiplying) and
                # the first output chunk overlap the second pass.
                for h in range(2):
                    for k in range(K):
                        t = tg * K + k
                        tmp = shpool.tile([128, HF], f16, tag="sh")
                        tv = tmp[:].rearrange("p (r c) -> p r c",
                                              r=PB // 2, c=PB)
                        wv = w_tiles[t][:].rearrange(
                            "p (r c) -> p r c", r=PB, c=PB)[
                            :, h * (PB // 2):(h + 1) * (PB // 2), :]
                        nc.vector.tensor_mul(
                            out=tv, in0=wv,
                            in1=fview(t, h * (PB // 2), PB // 2))
                        pe_acc(tmp, range(4 * h, 4 * h + 4))
                    if h == 0:
                        for b in range(4):
                            nc.scalar.copy(
                                out=osb[:][:, b * BANK:(b + 1) * BANK],
                                in_=banks[b][:])
                        nc.sync.dma_start(out=oout[:, :O_FREE // 2],
                                          in_=osb[:][:, :O_FREE // 2])

            def fp_chunk(par, s):
                a, b = FP_SPLITS[s] * PH, FP_SPLITS[s + 1] * PH
                nc.sync.dma_start(out=fp_t[par][:][:, a:b],
                                  in_=fpd[par][:, a:b])

            def wq_dma(t, q):
                wt = whpool.tile([128, QT], i8, tag=f"wq8{q}",
                                 name=f"w{t}q{q}")
                nc.sync.dma_start(
                    out=wt[:], in_=win[t][:, q * QT:(q + 1) * QT])
                return wt

            # ---- head: frame chunks + t0/t1 quarters interleaved so the
            # first quarter-mul starts when ~0.6 MB has landed ----
            wq = {}
            fp_chunk(0, 0)
            wq[(0, 0)] = wq_dma(0, 0)
            fp_chunk(0, 1)
            wq[(0, 1)] = wq_dma(0, 1)
            emit_w(0, 2)
            fp_chunk(0, 2)
            wq[(0, 2)] = wq_dma(0, 2)
            fp_chunk(0, 3)
            wq[(0, 3)] = wq_dma(0, 3)
            emit_w(0, 3)
            fp_chunk(1, 0)
            fp_chunk(1, 1)
            wq[(1, 0)] = wq_dma(1, 0)
            wq[(1, 1)] = wq_dma(1, 1)
            fp_chunk(1, 2)
            fp_chunk(1, 3)
            wq[(1, 2)] = wq_dma(1, 2)
            wq[(1, 3)] = wq_dma(1, 3)
            emit_w(1, 0)
            emit_w(1, 1)
            emit_w(0, 4)
            emit_w(1, 2)
            # identity rides the (empty) ACT hwdge ring, keeping its 128
            # tiny descriptors out of the sync ring's ramp window
            nc.scalar.dma_start(out=id_t[:], in_=ident[:])

            # group 0 compute: 20 quarter-muls chasing the chunk DMAs;
            # t0/t1 dequantized by DVE in its arrival-wait gaps
            for k in range(K):
                for q in range(4):
                    t = k
                    if k in (0, 1):
                        wf = whpool.tile([128, QT], f16, tag=f"wqf{q}",
                                         name=f"wf{t}q{q}")
                        nc.vector.tensor_copy(out=wf[:], in_=wq[(k, q)][:])
                        wv = wf[:].rearrange(
                            "p (r c) -> p r c", r=PB // 4, c=PB)
                    else:
                        wv = w_tiles[k][:].rearrange(
                            "p (r c) -> p r c", r=PB, c=PB)[
                            :, q * (PB // 4):(q + 1) * (PB // 4), :]
                    tmp = shpool.tile([128, QT], f16, tag="sq")
                    tv = tmp[:].rearrange("p (r c) -> p r c",
                                          r=PB // 4, c=PB)
                    nc.vector.tensor_mul(
                        out=tv, in0=wv,
                        in1=fview(t, q * (PB // 4), PB // 4))
                    pe_acc(tmp, [2 * q, 2 * q + 1])

            # steady groups; weight DMAs in deadline order
            emit_w(2, 0)
            emit_w(1, 3)
            emit_w(2, 1)
            emit_w(1, 4)
            emit_w(2, 2)
            emit_compute(1)
            emit_w(3, 0)
            emit_w(2, 3)
            emit_w(3, 1)
            emit_w(2, 4)
            emit_w(3, 2)
            emit_compute(2)
            emit_w(4, 0)
            emit_w(3, 3)
            emit_w(4, 1)
            emit_w(3, 4)
            emit_w(4, 2)
            emit_compute(3)
            emit_w(4, 3)
            emit_w(4, 4)
            emit_compute_last(K - 1)

            # ---- tail: banks 4-7 evac split scalar/vector, output in
            # two quarter chunks ----
            for b in range(4, NBANK):
                dst = osb[:][:, b * BANK:(b + 1) * BANK]
                if b % 2 == 0:
                    nc.scalar.copy(out=dst, in_=banks[b][:])
                else:
                    nc.vector.tensor_copy(out=dst, in_=banks[b][:])
                if b == 5:
                    nc.sync.dma_start(
                        out=oout[:, 4 * BANK:6 * BANK],
                        in_=osb[:][:, 4 * BANK:6 * BANK])
            nc.sync.dma_start(out=oout[:, 6 * BANK:],
                              in_=osb[:][:, 6 * BANK:])
    nc.finalize()
    return nc


def _host_prep(frames, core):
    """Build per-core in_maps. frames [4,4,1,512,512] f32, core [4,4,25,1,512,512]."""
    G = NCORES * IMGS_PER_CORE  # 16
    F = np.ascontiguousarray(frames.reshape(G, H, W))
    Wc = core.reshape(G, K * K, H, W)

    # frame patches, 2 column-parity copies with 2-pixel halo
    Fp = np.pad(F, ((0, 0), (2, 2), (2, 3))).astype(np.float16)
    fpd = np.empty((G, 2, PG * PG, PH, PH), np.float16)
    for par in range(2):
        for pr in range(PG):
            for pc in range(PG):
                fpd[:, par, pr * PG + pc] = Fp[
                    :, pr * PB:pr * PB + PH,
                    pc * PB + par:pc * PB + par + PH]
    fprep = np.ascontiguousarray(
        fpd.reshape(NCORES, IMGS_PER_CORE, 2, PG * PG, PATCH)
        .transpose(0, 2, 1, 3, 4))  # [core, par, img, patch, PATCH]

    # weights, patch-major codes of w/2^-5
    def patchify(a):  # [G, T, 512, 512] -> [core, T, 128, 4096]
        T = a.shape[1]
        b = (a.reshape(G, T, PG, PB, PG, PB)
             .transpose(0, 1, 2, 4, 3, 5)
             .reshape(NCORES, IMGS_PER_CORE, T, PG * PG, FREE)
             .transpose(0, 2, 1, 3, 4))
        return np.ascontiguousarray(b.reshape(NCORES, T, 128, FREE))

    w8 = np.clip(np.round(Wc * (1.0 / WSCALE)), -127, 127).astype(np.int8)
    wprep = patchify(w8)
    tf = list(FP16_W)
    wfprep = patchify((Wc[:, tf] * (1.0 / WSCALE)).astype(np.float16))

    ident = (np.eye(128) * WSCALE).astype(np.float16)
    in_maps = []
    for c in range(NCORES):
        in_maps.append({
            "ident": ident,
            "fpd": fprep[c].reshape(2, 128, PATCH),
            "win": wprep[c],
            "win16": wfprep[c],
        })
    return in_maps


def kernel(frames, core, bias):
    global last_results
    from concourse.bass_utils import run_bass_kernel_spmd

    frames = np.asarray(frames, dtype=np.float32)
    core = np.asarray(core, dtype=np.float32)

    if "nc" not in _compiled:
        _compiled["nc"] = _build_nc()
    nc = _compiled["nc"]

    in_maps = _host_prep(frames, core)
    trace = os.environ.get("KC_TRACE") == "1"
    tmpdir = os.environ.get("KC_TRACE_DIR") or None
    if tmpdir:
        os.makedirs(tmpdir, exist_ok=True)
    res = run_bass_kernel_spmd(nc, in_maps, list(range(NCORES)), trace=trace,
                               tmpdir=tmpdir)
    last_results = res

    G = NCORES * IMGS_PER_CORE
    out = np.empty((G, H, W), np.float32)
    for c in range(NCORES):
        o = res.results[c]["oout"]  # [128, 4096] f16
        ov = (o.reshape(IMGS_PER_CORE, PG, PG, PB, PB).astype(np.float32)
              .transpose(0, 1, 3, 2, 4).reshape(IMGS_PER_CORE, H, W))
        for img in range(IMGS_PER_CORE):
            out[c * IMGS_PER_CORE + img] = ov[img]
    return out.reshape(4, 4, H, W)


# revision 28
# speedup vs baseline: 1.1776x; 1.0067x over previous
"""Per-pixel adaptive 5x5 conv (KPN) for Trainium2, 8-core data parallel.

out[g,h,w] = sum_{i,j} core[g,5i+j,h,w] * frames_pad[g,h+i-2,w+j-2]
with g = flattened (B,N) = 16 image planes; 2 planes per NeuronCore.

Layout: PARTITION = 64x64 IMAGE PATCH (8x8 grid x 2 images = 128
partitions).  Each partition's free dim holds its patch row-major, so
BOTH tap shifts (i,j) become free-dim offsets into a single halo'd
frame tile fpd[par][p, 68*68]: value (R,C) = f[pr*64+R-2, pc*64+C-2+par].
Two column-parity copies (par=0 for even j, par=1 for odd j) keep every
tap's in1 access 4-byte aligned for DVE fp16 2x mode.  Frames cost
2*1.18 MB of DMA total (the v1 row-block layout needed 5 row-shifted
copies = 10.6 MB; v2 single-parity still needed 5.3 MB).  Weights are
compact [128, 64*64] patch-major, no padding.

Engine split (balanced so DVE's 25 muls, ACT's cast chain, and the DMA
queues all land at ~55-60us):
  DVE   - 25 products w_t * f_shifted (fp16 2x, ~2.2us full tile) plus
          t0/t1 quarter dequants in the ramp's arrival-wait gaps
  ACT   - 13 weight dequants (int8 codes -> fp16 copy, 3.6us each)
  DMA   - 10 weight tiles stored as fp16 codes in DRAM, loaded directly
          (no dequant anywhere; +0.5 MB DMA each is cheaper than an
          engine cast while the queues have slack; SWDGE casting DMA
          costs ~read+write on the queues, worse than both)
  PE    - accumulates the 25 product streams into PSUM fp32 via
          matmuls against a stationary (2^-5 * I); 8 banks = [128,512]
          (bank b = patch rows 8b..8b+8)

Scheduling (DMA queues round-robin ALL in-flight transfers at packet
granularity, so any tile's completion time scales with total bytes in
flight — ordering is everything):
  - weight DMAs in consumption-deadline order: int8 (ACT-cast) tiles
    ~a group ahead of fp16-direct tiles (their arrival is followed by
    a serial 3.6us ACT cast before the mul can run)
  - frame patches arrive as 4 row-chunk DMAs into one tile per parity,
    and group 0 runs as [128,16x64] quarter muls chasing the chunks
  - last group runs in per-image-half passes: banks 0-3 get their
    final accumulation a pass early, so their evac (on the
    then-idle ACT) and the first 0.5 MB output store overlap the
    last five muls

Weights are codes of w/2^-5 (int8: clip(round(32w), -127, 127); fp16
taps keep full precision); the 2^-5 scale is folded into the PE's
stationary identity, so every dequant is a pure cast.

Host layouts (per core, p = img*64 + pr*8 + pc):
  fpd [2, 128, 4624] fp16: fpd[par][p, R*68+C] =
      Fpad[img, pr*64+R, pc*64+C+par], Fpad = pad(F, rows 2/2, cols 2/3)
  win [25, 128, 4096] int8, win16 [11, 128, 4096] fp16:
      [t][p, Rp*64+Cp] = codes of core[img, t, pr*64+Rp, pc*64+Cp]
  oout [128, 4096] fp16 (host casts to f32 and reassembles patches).
"""

import os
import sys

import numpy as np

for _p in ("/opt/trn_rl_repo",):
    if _p not in sys.path and os.path.isdir(_p):
        sys.path.insert(0, _p)

K = 5
NCORES = 8
IMGS_PER_CORE = 2
H = W = 512
PB = 64           # patch edge
PG = 8            # patch grid (8x8 per image)
PH = PB + 4       # halo'd patch edge = 68
PATCH = PH * PH   # 4624 elems per partition per parity
FREE = PB * PB    # 4096 product/weight elems per partition
O_FREE = FREE
NBANK = 8
BANK = 512        # = 8 patch rows
WSCALE = 2.0 ** -5  # weight dequant scale, folded into PE identity

# Dequant engine assignment per tap t = 5*i + j.  t0/t1 are int8
# quarter tiles cast by DVE in the ramp window (DVE idles on arrivals
# there anyway, and int8 halves their share of the head DMA flood).
FP16_W = (4, 8, 9, 13, 14, 18, 19, 22, 23, 24)  # stored fp16, no dequant
FP16_IDX = {t: n for n, t in enumerate(FP16_W)}
# remaining 13 taps dequant on ACT

# fpd row-chunk splits (halo rows): mul quarter q (out rows 16q..16q+16)
# needs patch rows [16q, 16q+20) at i<=4 => cumulative chunks cover it
FP_SPLITS = (0, 20, 36, 52, 68)

_compiled = {}
last_results = None  # BassKernelResults of the most recent run (for test.py)


def _build_nc():
    import concourse.bacc as bacc
    import concourse.mybir as mybir
    from concourse.tile import TileContext

    f16 = mybir.dt.float16
    f32 = mybir.dt.float32
    i8 = mybir.dt.int8

    nc = bacc.Bacc(None, target_bir_lowering=False, debug=False)
    ident = nc.dram_tensor("ident", [128, 128], f16, kind="ExternalInput")
    fpd = nc.dram_tensor("fpd", [2, 128, PATCH], f16, kind="ExternalInput")
    win = nc.dram_tensor("win", [K * K, 128, FREE], i8, kind="ExternalInput")
    win16 = nc.dram_tensor("win16", [len(FP16_W), 128, FREE], f16,
                           kind="ExternalInput")
    oout = nc.dram_tensor("oout", [128, O_FREE], f16, kind="ExternalOutput")

    n_streams = K * K
    QT = FREE // 4   # 1024 = quarter product (16 out rows)
    HF = FREE // 2   # 2048 = half product (32 out rows)

    with TileContext(nc) as tc:
        with (
            tc.tile_pool(name="ipool", bufs=1) as ipool,
            tc.tile_pool(name="fppool", bufs=1) as fppool,
            tc.tile_pool(name="w8pool", bufs=2) as w8pool,
            tc.tile_pool(name="whpool", bufs=1) as whpool,
            tc.tile_pool(name="wpool", bufs=2) as wpool,
            tc.tile_pool(name="spool", bufs=3) as spool,
            tc.tile_pool(name="shpool", bufs=3) as shpool,
            tc.tile_pool(name="opool", bufs=1) as opool,
            tc.tile_pool(name="ppool", bufs=1, space="PSUM") as ppool,
        ):
            id_t = ipool.tile([128, 128], f16, tag="ident")

            banks = [ppool.tile([128, BANK], f32, tag=f"b{b}",
                                name=f"bank{b}")
                     for b in range(NBANK)]
            osb = opool.tile([128, O_FREE], f16, tag="osb")

            fp_t = [fppool.tile([128, PATCH], f16, tag=f"fp{par}",
                                name=f"fp{par}")
                    for par in range(2)]

            w8_tiles = {}
            w_tiles = {}
            bank_n = [0] * NBANK

            def fview(t, r0, nr):
                # frame operand for tap t over out rows [r0, r0+nr):
                # parity copy j%2, rows i+r0.., cols j (even) / j-1 (odd)
                i, j = divmod(t, K)
                par = j & 1
                c0 = j - par
                v = fp_t[par][:].rearrange("p (r c) -> p r c", r=PH, c=PH)
                return v[:, i + r0:i + r0 + nr, c0:c0 + PB]

            def pe_acc(tile, bank_list):
                for lb, b in enumerate(bank_list):
                    s = bank_n[b]
                    bank_n[b] += 1
                    nc.tensor.matmul(
                        out=banks[b][:],
                        lhsT=id_t[:],
                        rhs=tile[:][:, lb * BANK:(lb + 1) * BANK],
                        start=(s == 0),
                        stop=(s == n_streams - 1),
                    )

            def emit_w(tg, k):
                t = tg * K + k
                if t in FP16_IDX:
                    # fp16 codes straight from DRAM, no dequant step
                    w_t = wpool.tile([128, FREE], f16, tag=f"w{k}",
                                     name=f"wf16_{t}")
                    nc.sync.dma_start(out=w_t[:], in_=win16[FP16_IDX[t]])
                    w_tiles[t] = w_t
                    return
                w8_t = w8pool.tile([128, FREE], i8, tag=f"w8{k}",
                                   name=f"w8_{t}")
                nc.sync.dma_start(out=w8_t[:], in_=win[t])
                w8_tiles[t] = w8_t
                # ACT dequant emitted with the DMA so ACT chases arrivals
                w_t = wpool.tile([128, FREE], f16, tag=f"w{k}",
                                 name=f"wdq{t}")
                nc.scalar.copy(out=w_t[:], in_=w8_t[:])
                w_tiles[t] = w_t

            def emit_compute(tg):
                for k in range(K):
                    t = tg * K + k
                    tmp = spool.tile([128, FREE], f16, tag="s")
                    tv = tmp[:].rearrange("p (r c) -> p r c", r=PB, c=PB)
                    wv = w_tiles[t][:].rearrange("p (r c) -> p r c",
                                                 r=PB, c=PB)
                    nc.vector.tensor_mul(out=tv, in0=wv,
                                         in1=fview(t, 0, PB))
                    pe_acc(tmp, range(NBANK))

            def emit_compute_last(tg):
                # per-image-half passes: banks 0-3 finish a pass early;
                # their evac (ACT only -- DVE is still multiplying) and
                # the first output chunk overlap the second pass.
                for h in range(2):
                    for k in range(K):
                        t = tg * K + k
                        tmp = shpool.tile([128, HF], f16, tag="sh")
                        tv = tmp[:].rearrange("p (r c) -> p r c",
                                              r=PB // 2, c=PB)
                        wv = w_tiles[t][:].rearrange(
                            "p (r c) -> p r c", r=PB, c=PB)[
                            :, h * (PB // 2):(h + 1) * (PB // 2), :]
                        nc.vector.tensor_mul(
                            out=tv, in0=wv,
                            in1=fview(t, h * (PB // 2), PB // 2))
                        pe_acc(tmp, range(4 * h, 4 * h + 4))
                    if h == 0:
                        for b in range(4):
                            nc.scalar.copy(
                                out=osb[:][:, b * BANK:(b + 1) * BANK],
                                in_=banks[b][:])
                        nc.sync.dma_start(out=oout[:, :O_FREE // 2],
                                          in_=osb[:][:, :O_FREE // 2])

            def fp_chunk(par, s):
                a, b = FP_SPLITS[s] * PH, FP_SPLITS[s + 1] * PH
                nc.sync.dma_start(out=fp_t[par][:][:, a:b],
                                  in_=fpd[par][:, a:b])

            def wq_dma(t, q):
                wt = whpool.tile([128, QT], i8, tag=f"wq8{q}",
                                 name=f"w{t}q{q}")
                nc.sync.dma_start(
                    out=wt[:], in_=win[t][:, q * QT:(q + 1) * QT])
                return wt

            # ---- head: frame chunks + t0/t1 quarters interleaved so the
            # first quarter-mul starts when ~0.6 MB has landed ----
            wq = {}
            fp_chunk(0, 0)
            wq[(0, 0)] = wq_dma(0, 0)
            fp_chunk(0, 1)
            wq[(0, 1)] = wq_dma(0, 1)
            emit_w(0, 2)
            fp_chunk(0, 2)
            wq[(0, 2)] = wq_dma(0, 2)
            fp_chunk(0, 3)
            wq[(0, 3)] = wq_dma(0, 3)
            emit_w(0, 3)
            fp_chunk(1, 0)
            fp_chunk(1, 1)
            wq[(1, 0)] = wq_dma(1, 0)
            wq[(1, 1)] = wq_dma(1, 1)
            fp_chunk(1, 2)
            fp_chunk(1, 3)
            wq[(1, 2)] = wq_dma(1, 2)
            wq[(1, 3)] = wq_dma(1, 3)
            emit_w(1, 0)
            emit_w(1, 1)
            emit_w(0, 4)
            emit_w(1, 2)
            # identity rides the (empty) ACT hwdge ring, keeping its 128
            # tiny descriptors out of the sync ring's ramp window
            nc.scalar.dma_start(out=id_t[:], in_=ident[:])

            # group 0 compute: 20 quarter-muls chasing the chunk DMAs;
            # t0/t1 dequantized by DVE in its arrival-wait gaps
            for k in range(K):
                for q in range(4):
                    t = k
                    if k in (0, 1):
                        wf = whpool.tile([128, QT], f16, tag=f"wqf{q}",
                                         name=f"wf{t}q{q}")
                        nc.vector.tensor_copy(out=wf[:], in_=wq[(k, q)][:])
                        wv = wf[:].rearrange(
                            "p (r c) -> p r c", r=PB // 4, c=PB)
                    else:
                        wv = w_tiles[k][:].rearrange(
                            "p (r c) -> p r c", r=PB, c=PB)[
                            :, q * (PB // 4):(q + 1) * (PB // 4), :]
                    tmp = shpool.tile([128, QT], f16, tag="sq")
                    tv = tmp[:].rearrange("p (r c) -> p r c",
                                          r=PB // 4, c=PB)
                    nc.vector.tensor_mul(
                        out=tv, in0=wv,
                        in1=fview(t, q * (PB // 4), PB // 4))
                    pe_acc(tmp, [2 * q, 2 * q + 1])

            # steady groups; weight DMAs in deadline order
            emit_w(2, 0)
            emit_w(1, 3)
            emit_w(2, 1)
            emit_w(1, 4)
            emit_w(2, 2)
            emit_compute(1)
            emit_w(3, 0)
            emit_w(2, 3)
            emit_w(3, 1)
            emit_w(2, 4)
            emit_w(3, 2)
            emit_compute(2)
            emit_w(4, 0)
            emit_w(3, 3)
            emit_w(4, 1)
            emit_w(3, 4)
            emit_w(4, 2)
            emit_compute(3)
            emit_w(4, 3)
            emit_w(4, 4)
            emit_compute_last(K - 1)

            # ---- tail: banks 4-7 evac split scalar/vector, output in
            # two quarter chunks ----
            for b in range(4, NBANK):
                dst = osb[:][:, b * BANK:(b + 1) * BANK]
                if b % 2 == 0:
                    nc.scalar.copy(out=dst, in_=banks[b][:])
                else:
                    nc.vector.tensor_copy(out=dst, in_=banks[b][:])
                if b == 5:
                    nc.sync.dma_start(
                        out=oout[:, 4 * BANK:6 * BANK],
                        in_=osb[:][:, 4 * BANK:6 * BANK])
            nc.sync.dma_start(out=oout[:, 6 * BANK:],
                              in_=osb[:][:, 6 * BANK:])
    nc.finalize()
    return nc


def _host_prep(frames, core):
    """Build per-core in_maps. frames [4,4,1,512,512] f32, core [4,4,25,1,512,512]."""
    G = NCORES * IMGS_PER_CORE  # 16
    F = np.ascontiguousarray(frames.reshape(G, H, W))
    Wc = core.reshape(G, K * K, H, W)

    # frame patches, 2 column-parity copies with 2-pixel halo
    Fp = np.pad(F, ((0, 0), (2, 2), (2, 3))).astype(np.float16)
    fpd = np.empty((G, 2, PG * PG, PH, PH), np.float16)
    for par in range(2):
        for pr in range(PG):
            for pc in range(PG):
                fpd[:, par, pr * PG + pc] = Fp[
                    :, pr * PB:pr * PB + PH,
                    pc * PB + par:pc * PB + par + PH]
    fprep = np.ascontiguousarray(
        fpd.reshape(NCORES, IMGS_PER_CORE, 2, PG * PG, PATCH)
        .transpose(0, 2, 1, 3, 4))  # [core, par, img, patch, PATCH]

    # weights, patch-major codes of w/2^-5
    def patchify(a):  # [G, T, 512, 512] -> [core, T, 128, 4096]
        T = a.shape[1]
        b = (a.reshape(G, T, PG, PB, PG, PB)
             .transpose(0, 1, 2, 4, 3, 5)
             .reshape(NCORES, IMGS_PER_CORE, T, PG * PG, FREE)
             .transpose(0, 2, 1, 3, 4))
        return np.ascontiguousarray(b.reshape(NCORES, T, 128, FREE))

    w8 = np.clip(np.round(Wc * (1.0 / WSCALE)), -127, 127).astype(np.int8)
    wprep = patchify(w8)
    tf = list(FP16_W)
    wfprep = patchify((Wc[:, tf] * (1.0 / WSCALE)).astype(np.float16))

    ident = (np.eye(128) * WSCALE).astype(np.float16)
    in_maps = []
    for c in range(NCORES):
        in_maps.append({
            "ident": ident,
            "fpd": fprep[c].reshape(2, 128, PATCH),
            "win": wprep[c],
            "win16": wfprep[c],
        })
    return in_maps


def kernel(frames, core, bias):
    global last_results
    from concourse.bass_utils import run_bass_kernel_spmd

    frames = np.asarray(frames, dtype=np.float32)
    core = np.asarray(core, dtype=np.float32)

    if "nc" not in _compiled:
        _compiled["nc"] = _build_nc()
    nc = _compiled["nc"]

    in_maps = _host_prep(frames, core)
    trace = os.environ.get("KC_TRACE") == "1"
    tmpdir = os.environ.get("KC_TRACE_DIR") or None
    if tmpdir:
        os.makedirs(tmpdir, exist_ok=True)
    res = run_bass_kernel_spmd(nc, in_maps, list(range(NCORES)), trace=trace,
                               tmpdir=tmpdir)
    last_results = res

    G = NCORES * IMGS_PER_CORE
    out = np.empty((G, H, W), np.float32)
    for c in range(NCORES):
        o = res.results[c]["oout"]  # [128, 4096] f16
        ov = (o.reshape(IMGS_PER_CORE, PG, PG, PB, PB).astype(np.float32)
              .transpose(0, 1, 3, 2, 4).reshape(IMGS_PER_CORE, H, W))
        for img in range(IMGS_PER_CORE):
            out[c * IMGS_PER_CORE + img] = ov[img]
    return out.reshape(4, 4, H, W)
